# revision 1
# baseline (speedup 1.0000x reference)
"""Dense MoE (all-experts, gate-weighted sum) on 8 Trainium2 NeuronCores.

Sharding: pure data-parallel over the token axis N (8192 -> 1024 rows/core);
every core holds all 8 experts, so no collectives are needed.

Math folded per core (N_loc=1024, D=1024, E=8, O=1024, H=256):
    h      = relu(x @ W_g1.T + b_g1)                 # gating MLP, bf16 matmuls
    gates  = softmax(h @ W_g2.T + b_g2)              # fp32 softmax
    out    = sum_e gates[:,e] * (x @ W_e[e].T) + gates @ b_e

The expert GEMMs accumulate over D in PSUM (fp32); the gate weighting + sum
over experts is ACT mul (per-partition gate scale) + DVE add per tile.
The bias term rides a tiny K=8 matmul (gates.T as stationary operand),
overlapped with expert 1's GEMM stream.

All matmul operands are bf16 (host-cast); accumulation fp32. Measured on
hardware: ~265 us per core (PE-bound; bf16 N=512 matmul roofline is ~221 us),
rel err vs fp32 reference ~2.3e-3 absmax-relative.
"""

import numpy as np
import ml_dtypes

import concourse.bass as bass
import concourse.mybir as mybir
import concourse.tile as tile
from concourse.bass_utils import run_bass_kernel_spmd

N, D, E, O, H = 8192, 1024, 8, 1024, 256
NCORES = 8
NLOC = N // NCORES          # 1024 rows per core
P = 128                     # partitions
NT = NLOC // P              # 8 n-tiles
DK = D // P                 # 8 contraction tiles
FO = 512                    # matmul moving free dim (one PSUM bank of fp32)
OH = O // FO                # 2 output halves
H2 = H // P                 # 2 h-tiles
BF16 = mybir.dt.bfloat16
F32 = mybir.dt.float32
BF = ml_dtypes.bfloat16


def legalize_single_wait(nc, max_waits=1):
    """This walrus build rejects instructions carrying more than one sync
    wait. Split each multi-wait instruction: excess waits move onto fresh
    same-engine NoOps inserted immediately before it (identical semantics:
    the engine stalls at the same program point on every semaphore)."""
    for f in nc.m.functions:
        for blk in f.blocks:
            insts = list(blk.instructions)
            if all(
                (i.sync_info is None or len(i.sync_info.on_wait) <= max_waits)
                for i in insts
            ):
                continue
            new = []
            for inst in insts:
                si = inst.sync_info
                if si is not None and len(si.on_wait) > max_waits:
                    waits = list(si.on_wait)
                    for k, w in enumerate(waits[:-max_waits]):
                        nop = mybir.InstNoOp(name=f"{inst.name}-w{k}")
                        nop.engine = inst.engine
                        nop.sync_info = mybir.SyncInfo(on_wait=[w], on_update=[])
                        new.append(nop)
                    si.on_wait = waits[-max_waits:]
                new.append(inst)
            blk.instructions = new
    return nc


def build_moe():
    nc = bass.Bass(target_bir_lowering=False)
    xT = nc.dram_tensor("xT", [D, NLOC], BF16, kind="ExternalInput")
    wt = nc.dram_tensor("wt", [E, D, O], BF16, kind="ExternalInput")
    wg1t = nc.dram_tensor("wg1t", [D, H], BF16, kind="ExternalInput")
    wg2t = nc.dram_tensor("wg2t", [H, E], BF16, kind="ExternalInput")
    bg1 = nc.dram_tensor("bg1", [H], F32, kind="ExternalInput")
    bg2 = nc.dram_tensor("bg2", [E], BF16, kind="ExternalInput")
    be = nc.dram_tensor("be", [E, O], BF16, kind="ExternalInput")
    ident = nc.dram_tensor("ident", [P, P], F32, kind="ExternalInput")
    out = nc.dram_tensor("out", [NLOC, O], F32, kind="ExternalOutput")

    with tile.TileContext(nc) as tc:
        with (
            tc.tile_pool(name="const", bufs=1) as constp,
            tc.tile_pool(name="wpool", bufs=4) as wpool,
            tc.tile_pool(name="work", bufs=4) as workp,
            tc.tile_pool(name="pro_ps", bufs=2, space="PSUM") as prop,
            tc.tile_pool(name="bias_ps", bufs=1, space="PSUM") as biasp,
            tc.tile_pool(name="mm_ps", bufs=5, space="PSUM") as mmp,
        ):
            # ---- PE warm-up: dummy matmuls on memset tiles (no DMA deps)
            # keep the PE busy while the first transfers land, so the HAM
            # clock-gate reaches 2.4 GHz before real work arrives ----
            warm_a = constp.tile([P, P], BF16, tag="warm_a")
            nc.vector.memset(warm_a, 0.0)
            warm_b = constp.tile([P, FO], BF16, tag="warm_b")
            nc.vector.memset(warm_b, 0.0)
            for i in range(24):
                wpsum = mmp.tile([P, FO], F32, tag="mm", name=f"warm{i}")
                nc.tensor.matmul(wpsum, warm_a, warm_b, start=True, stop=True)

            # ---- resident inputs (gating-critical transfers first, per-dk
            # interleaved so the first gating matmuls start ASAP) ----
            wg1t_sb = [
                constp.tile([P, H], BF16, tag=f"wg1t{dk}", name=f"wg1t{dk}")
                for dk in range(DK)
            ]
            xT_sb = [
                constp.tile([P, NLOC], BF16, tag=f"xTd{dk}", name=f"xTd{dk}")
                for dk in range(DK)
            ]
            wt0_r = wt[0].rearrange("(dk p) o -> p dk o", p=P)
            w0_half = []
            for oh in range(OH):
                wh = wpool.tile([P, DK, FO], BF16, tag=f"wh{oh}", name=f"wh{oh}")
                w0_half.append(wh)
            for dk in range(DK):
                nc.sync.dma_start(
                    out=wg1t_sb[dk], in_=wg1t[dk * P : (dk + 1) * P, :]
                )
                nc.sync.dma_start(
                    out=xT_sb[dk], in_=xT[dk * P : (dk + 1) * P, :]
                )
                nc.sync.dma_start(
                    out=w0_half[0][:, dk, :], in_=wt0_r[:, dk, 0:FO]
                )
            nc.sync.dma_start(out=w0_half[1], in_=wt0_r[:, :, FO : 2 * FO])
            wg2t_sb = constp.tile([P, H2, E], BF16, tag="wg2t")
            nc.gpsimd.dma_start(
                out=wg2t_sb, in_=wg2t.rearrange("(h2 p) e -> p h2 e", p=P)
            )
            bg1_sb = constp.tile([P, H2], F32, tag="bg1")
            nc.gpsimd.dma_start(out=bg1_sb, in_=bg1.rearrange("(h2 p) -> p h2", p=P))
            bg2_sb = constp.tile([1, E], BF16, tag="bg2")
            nc.gpsimd.dma_start(out=bg2_sb, in_=bg2[:])
            be_sb = constp.tile([E, O], BF16, tag="be")
            nc.gpsimd.dma_start(out=be_sb, in_=be[:, :])
            ones_sb = constp.tile([1, P], BF16, tag="ones")
            nc.vector.memset(ones_sb, 1.0)
            ident_sb = constp.tile([P, P], F32, tag="ident")
            nc.gpsimd.dma_start(out=ident_sb, in_=ident[:, :])

            # ---- gating: hT[h, n] = relu(W_g1 @ x.T + b_g1) ----
            hT_sb = [
                constp.tile([P, NLOC], BF16, tag=f"hT{h2}", name=f"hT{h2}") for h2 in range(H2)
            ]
            psum_g = {
                (h2, nh): mmp.tile([P, FO], F32, tag="mm", name=f"psum_g{h2}_{nh}")
                for h2 in range(H2)
                for nh in range(NLOC // FO)
            }
            for dk in range(DK):
                for h2 in range(H2):
                    for nh in range(NLOC // FO):
                        nc.tensor.matmul(
                            psum_g[(h2, nh)],
                            wg1t_sb[dk][:, h2 * P : (h2 + 1) * P],
                            xT_sb[dk][:, nh * FO : (nh + 1) * FO],
                            start=(dk == 0),
                            stop=(dk == DK - 1),
                        )
            for h2 in range(H2):
                for nh in range(NLOC // FO):
                    nc.scalar.activation(
                        out=hT_sb[h2][:, nh * FO : (nh + 1) * FO],
                        in_=psum_g[(h2, nh)],
                        func=mybir.ActivationFunctionType.Relu,
                        bias=bg1_sb[:, h2 : h2 + 1],
                    )

            # ---- gating: logits -> softmax -> gates, gates.T ----
            gates_sb = []
            gatesT_sb = []
            for nt in range(NT):
                psum_l = prop.tile([P, E], F32, tag="pro")
                for h2 in range(H2):
                    nc.tensor.matmul(
                        psum_l,
                        hT_sb[h2][:, nt * P : (nt + 1) * P],
                        wg2t_sb[:, h2, :],
                        start=(h2 == 0),
                        stop=False,
                    )
                nc.tensor.matmul(psum_l, ones_sb, bg2_sb, start=False, stop=True)

                negmax = workp.tile([P, 1], F32, tag="negmax")
                nc.vector.reduce_max(
                    negmax, psum_l, axis=mybir.AxisListType.X, negate=True
                )
                gates = constp.tile([P, E], F32, tag=f"gates{nt}", name=f"gates{nt}")
                sumexp = workp.tile([P, 1], F32, tag="sumexp")
                nc.scalar.activation(
                    out=gates,
                    in_=psum_l,
                    func=mybir.ActivationFunctionType.Exp,
                    bias=negmax,
                    accum_out=sumexp,
                )
                rsum = workp.tile([P, 1], F32, tag="rsum")
                nc.vector.reciprocal(rsum, sumexp)
                nc.vector.tensor_scalar_mul(gates, gates, rsum)
                gates_sb.append(gates)

            acc_sb = [
                [
                    constp.tile(
                        [P, FO], F32, tag=f"acc{nt}_{oh}", name=f"acc{nt}_{oh}"
                    )
                    for oh in range(OH)
                ]
                for nt in range(NT)
            ]

            # ---- main loop: stream experts, accumulate gate-weighted GEMM ----
            for e in range(E):
                if e == 0:
                    w_half = w0_half
                else:
                    wt_r = wt[e].rearrange("(dk p) o -> p dk o", p=P)
                    w_half = []
                    for oh in range(OH):
                        wh = wpool.tile(
                            [P, DK, FO], BF16, tag=f"wh{oh}", name=f"wh{oh}"
                        )
                        nc.sync.dma_start(
                            out=wh, in_=wt_r[:, :, oh * FO : (oh + 1) * FO]
                        )
                        w_half.append(wh)
                for oh in range(OH):
                    for nt in range(NT):
                        psum = mmp.tile([P, FO], F32, tag="mm")
                        for dk in range(DK):
                            nc.tensor.matmul(
                                psum,
                                xT_sb[dk][:, nt * P : (nt + 1) * P],
                                w_half[oh][:, dk, :],
                                start=(dk == 0),
                                stop=(dk == DK - 1),
                            )
                        acc = acc_sb[nt][oh]
                        if e == 0:
                            nc.scalar.mul(acc, psum, gates_sb[nt][:, e : e + 1])
                        else:
                            tmp = workp.tile([P, FO], F32, tag="tmp", name="tmp")
                            nc.scalar.mul(tmp, psum, gates_sb[nt][:, e : e + 1])
                            nc.vector.tensor_add(acc, acc, tmp)
                        if e == E - 1:
                            nc.scalar.dma_start(
                                out=out[
                                    nt * P : (nt + 1) * P, oh * FO : (oh + 1) * FO
                                ],
                                in_=acc,
                            )
                if e == 0:
                    # gates.T + bias matmuls — emitted here so the PE work
                    # hides inside experts 0-1's dense matmul stream and the
                    # kernel tail stays short
                    for nt in range(NT):
                        psum_t = prop.tile([E, P], F32, tag="pro", name="psum_t")
                        nc.tensor.transpose(psum_t, gates_sb[nt], ident_sb)
                        gatesT = constp.tile(
                            [E, P], BF16, tag=f"gatesT{nt}", name=f"gatesT{nt}"
                        )
                        nc.scalar.copy(out=gatesT, in_=psum_t)
                        gatesT_sb.append(gatesT)
                if e == 1:
                    for nt in range(NT):
                        for boh in range(OH):
                            psum_b = biasp.tile(
                                [P, FO], F32, tag="bias", name="psum_b"
                            )
                            nc.tensor.matmul(
                                psum_b,
                                gatesT_sb[nt],
                                be_sb[:, boh * FO : (boh + 1) * FO],
                                start=True,
                                stop=True,
                            )
                            nc.vector.tensor_add(
                                acc_sb[nt][boh], acc_sb[nt][boh], psum_b
                            )

    legalize_single_wait(nc)
    return nc


_NC_CACHE = {}


def _get_nc():
    if "nc" not in _NC_CACHE:
        _NC_CACHE["nc"] = build_moe()
    return _NC_CACHE["nc"]


def make_in_maps(x, W_e, b_e, W_g1, b_g1, W_g2, b_g2):
    x = np.asarray(x, dtype=np.float32)
    wt = np.ascontiguousarray(
        np.asarray(W_e, dtype=np.float32).transpose(0, 2, 1)
    ).astype(BF)
    wg1t = np.ascontiguousarray(np.asarray(W_g1, dtype=np.float32).T).astype(BF)
    wg2t = np.ascontiguousarray(np.asarray(W_g2, dtype=np.float32).T).astype(BF)
    bg1 = np.asarray(b_g1, dtype=np.float32)
    bg2 = np.asarray(b_g2, dtype=np.float32).astype(BF)
    be = np.asarray(b_e, dtype=np.float32).astype(BF)
    xb = x.astype(BF)
    ident_np = np.eye(P, dtype=np.float32)
    in_maps = []
    for c in range(NCORES):
        xT_c = np.ascontiguousarray(xb[c * NLOC : (c + 1) * NLOC, :].T)
        in_maps.append(
            {
                "xT": xT_c,
                "wt": wt,
                "wg1t": wg1t,
                "wg2t": wg2t,
                "bg1": bg1,
                "bg2": bg2,
                "be": be,
                "ident": ident_np,
            }
        )
    return in_maps


def kernel(x, W_e, b_e, W_g1, b_g1, W_g2, b_g2, **run_kwargs):
    nc = _get_nc()
    in_maps = make_in_maps(x, W_e, b_e, W_g1, b_g1, W_g2, b_g2)
    res = run_bass_kernel_spmd(nc, in_maps, core_ids=list(range(NCORES)), **run_kwargs)
    out = np.concatenate([res.results[c]["out"] for c in range(NCORES)], axis=0)
    if run_kwargs:
        kernel.last_results = res
    return out


if __name__ == "__main__":
    rng = np.random.default_rng(0)
    s = 1.0 / np.sqrt(D)
    sh = 1.0 / np.sqrt(H)
    inputs = {
        "x": rng.standard_normal((N, D), dtype=np.float32),
        "W_e": rng.uniform(-s, s, (E, O, D)).astype(np.float32),
        "b_e": rng.uniform(-s, s, (E, O)).astype(np.float32),
        "W_g1": rng.uniform(-s, s, (H, D)).astype(np.float32),
        "b_g1": rng.uniform(-s, s, (H,)).astype(np.float32),
        "W_g2": rng.uniform(-sh, sh, (E, H)).astype(np.float32),
        "b_g2": rng.uniform(-sh, sh, (E,)).astype(np.float32),
    }
    out = kernel(**inputs)
    print("out", out.shape, out.dtype, float(np.abs(out).max()))



# revision 2
# speedup vs baseline: 1.0711x; 1.0711x over previous
"""Dense MoE (all-experts, gate-weighted sum) on 8 Trainium2 NeuronCores.

Sharding: pure data-parallel over the token axis N (8192 -> 1024 rows/core);
every core holds all 8 experts, so no collectives are needed.

Math folded per core (N_loc=1024, D=1024, E=8, O=1024, H=256):
    h      = relu(x @ W_g1.T + b_g1)                 # gating MLP
    gates  = softmax(h @ W_g2.T + b_g2)              # fp32 softmax
    out    = sum_e gates[:,e] * (x @ W_e[e].T) + gates @ b_e

Precision/speed hybrid: the expert GEMM contraction over D=1024 runs
dk 0-5 (768 rows) in bf16 and dk 6-7 (256 rows) as ONE fp8-e4m3
DoubleRow matmul (2x PE rate), all accumulating in the same fp32 PSUM
bank. To share one PSUM scale, x is pre-scaled by 32 for the bf16 path
(matching W8 = e4m3(32*W) on the fp8 path) and the gating network sees
the same 32x through an exact rescale: b_g1 *= 32 (relu is positively
homogeneous) and W_g2 /= 32, so logits/gates are unchanged. The gate
weighting uses gates/32 to undo the scale.

The bias term rides a tiny K=8 matmul (gates.T as stationary operand),
overlapped with expert 1's GEMM stream.

Input DMAs are split across the sync-engine and gpsimd-engine queues
(parallel hardware rings) with host-side swizzles giving 2-12KB
contiguous lines per partition, so the gating + expert-0 operands land
before the PE finishes its warm-up/gating phase (avoids the mid-kernel
HAM half-clock dip the serial-queue version hit).
"""

import numpy as np
import ml_dtypes

import concourse.bass as bass
import concourse.mybir as mybir
import concourse.tile as tile
from concourse.bass_utils import run_bass_kernel_spmd

N, D, E, O, H = 8192, 1024, 8, 1024, 256
NCORES = 8
NLOC = N // NCORES          # 1024 rows per core
P = 128                     # partitions
NT = NLOC // P              # 8 n-tiles
DK = D // P                 # 8 contraction tiles
DK6 = 6                     # bf16 contraction tiles (dk 0-5)
KCUT = DK6 * P              # 768: d >= KCUT handled by the fp8 pair
FO = 512                    # matmul moving free dim (one PSUM bank of fp32)
OH = O // FO                # 2 output halves
H2 = H // P                 # 2 h-tiles
SCALE = 32.0
BF16 = mybir.dt.bfloat16
FP8 = mybir.dt.float8e4
F32 = mybir.dt.float32
BF = ml_dtypes.bfloat16
E4M3 = ml_dtypes.float8_e4m3
NWARM = 16


def legalize_single_wait(nc, max_waits=1):
    """This walrus build rejects instructions carrying more than one sync
    wait. Split each multi-wait instruction: excess waits move onto fresh
    same-engine NoOps inserted immediately before it (identical semantics:
    the engine stalls at the same program point on every semaphore)."""
    for f in nc.m.functions:
        for blk in f.blocks:
            insts = list(blk.instructions)
            if all(
                (i.sync_info is None or len(i.sync_info.on_wait) <= max_waits)
                for i in insts
            ):
                continue
            new = []
            for inst in insts:
                si = inst.sync_info
                if si is not None and len(si.on_wait) > max_waits:
                    waits = list(si.on_wait)
                    for k, w in enumerate(waits[:-max_waits]):
                        nop = mybir.InstNoOp(name=f"{inst.name}-w{k}")
                        nop.engine = inst.engine
                        nop.sync_info = mybir.SyncInfo(on_wait=[w], on_update=[])
                        new.append(nop)
                    si.on_wait = waits[-max_waits:]
                new.append(inst)
            blk.instructions = new
    return nc


def build_moe():
    nc = bass.Bass(target_bir_lowering=False)
    xT = nc.dram_tensor("xT", [D, NLOC], BF16, kind="ExternalInput")      # 32*x.T
    x8 = nc.dram_tensor("x8", [P, 2, NLOC], FP8, kind="ExternalInput")    # x pair
    wt6 = nc.dram_tensor("wt6", [E, P, DK6, O], BF16, kind="ExternalInput")
    w8 = nc.dram_tensor("w8", [E, P, 2, O], FP8, kind="ExternalInput")    # 32*W
    wg1t = nc.dram_tensor("wg1t", [D, H], BF16, kind="ExternalInput")
    wg2t = nc.dram_tensor("wg2t", [H, E], BF16, kind="ExternalInput")     # /32
    bg1 = nc.dram_tensor("bg1", [H], F32, kind="ExternalInput")           # *32
    bg2 = nc.dram_tensor("bg2", [E], BF16, kind="ExternalInput")
    be = nc.dram_tensor("be", [E, O], BF16, kind="ExternalInput")
    ident = nc.dram_tensor("ident", [P, P], F32, kind="ExternalInput")
    out = nc.dram_tensor("out", [NLOC, O], F32, kind="ExternalOutput")

    with tile.TileContext(nc) as tc:
        with (
            tc.tile_pool(name="const", bufs=1) as constp,
            tc.tile_pool(name="wpool", bufs=4) as wpool,
            tc.tile_pool(name="work", bufs=4) as workp,
            tc.tile_pool(name="pro_ps", bufs=2, space="PSUM") as prop,
            tc.tile_pool(name="bias_ps", bufs=1, space="PSUM") as biasp,
            tc.tile_pool(name="mm_ps", bufs=5, space="PSUM") as mmp,
        ):
            # ---- PE warm-up: dummy matmuls on memset tiles (no DMA deps)
            # keep the PE busy while the first transfers land, so the HAM
            # clock-gate reaches 2.4 GHz before real work arrives ----
            warm_a = constp.tile([P, P], BF16, tag="warm_a")
            nc.vector.memset(warm_a, 0.0)
            warm_b = constp.tile([P, FO], BF16, tag="warm_b")
            nc.vector.memset(warm_b, 0.0)
            for i in range(NWARM):
                wpsum = mmp.tile([P, FO], F32, tag="mm", name=f"warm{i}")
                nc.tensor.matmul(wpsum, warm_a, warm_b, start=True, stop=True)

            # ---- resident inputs. Gating-critical chunks (wg1t, xT) are
            # interleaved per-dk and split across BOTH dma queues (sync gets
            # even dk, gpsimd odd dk) so the gating matmul stream is paced at
            # twice the single-queue delivery rate. Expert-0 weights ride the
            # gpsimd queue, experts 1-7 the sync queue. ----
            wg1t_sb = [
                constp.tile([P, H], BF16, tag=f"wg1t{dk}", name=f"wg1t{dk}")
                for dk in range(DK)
            ]
            xT_sb = [
                constp.tile([P, NLOC], BF16, tag=f"xTd{dk}", name=f"xTd{dk}")
                for dk in range(DK)
            ]
            for dk in range(0, DK, 2):
                nc.sync.dma_start(
                    out=wg1t_sb[dk], in_=wg1t[dk * P : (dk + 1) * P, :]
                )
                nc.gpsimd.dma_start(
                    out=wg1t_sb[dk + 1], in_=wg1t[(dk + 1) * P : (dk + 2) * P, :]
                )
                nc.sync.dma_start(out=xT_sb[dk], in_=xT[dk * P : (dk + 1) * P, :])
                nc.gpsimd.dma_start(
                    out=xT_sb[dk + 1], in_=xT[(dk + 1) * P : (dk + 2) * P, :]
                )
            # small gating/bias constants (tiny, needed ~20us in)
            wg2t_sb = constp.tile([P, H2, E], BF16, tag="wg2t")
            nc.gpsimd.dma_start(
                out=wg2t_sb, in_=wg2t.rearrange("(h2 p) e -> p h2 e", p=P)
            )
            bg1_sb = constp.tile([P, H2], F32, tag="bg1")
            nc.gpsimd.dma_start(out=bg1_sb, in_=bg1.rearrange("(h2 p) -> p h2", p=P))
            bg2_sb = constp.tile([1, E], BF16, tag="bg2")
            nc.gpsimd.dma_start(out=bg2_sb, in_=bg2[:])
            ident_sb = constp.tile([P, P], F32, tag="ident")
            nc.gpsimd.dma_start(out=ident_sb, in_=ident[:, :])
            # expert-0 weights on the gpsimd queue (12KB/2KB lines)
            w0_6 = wpool.tile([P, DK6, O], BF16, tag="wh6", name="wh6_e0")
            nc.gpsimd.dma_start(out=w0_6, in_=wt6[0])
            w0_8 = wpool.tile([P, 2, O], FP8, tag="wh8", name="wh8_e0")
            nc.gpsimd.dma_start(out=w0_8, in_=w8[0])
            x8_sb = constp.tile([P, 2, NLOC], FP8, tag="x8")
            nc.gpsimd.dma_start(out=x8_sb, in_=x8[:, :, :])
            be_sb = constp.tile([E, O], BF16, tag="be")
            nc.gpsimd.dma_start(out=be_sb, in_=be[:, :])
            ones_sb = constp.tile([1, P], BF16, tag="ones")
            nc.vector.memset(ones_sb, 1.0)

            # ---- gating: hT[h, n] = relu(W_g1 @ (32x).T + 32*b_g1) = 32*h ----
            hT_sb = [
                constp.tile([P, NLOC], BF16, tag=f"hT{h2}", name=f"hT{h2}")
                for h2 in range(H2)
            ]
            psum_g = {
                (h2, nh): mmp.tile([P, FO], F32, tag="mm", name=f"psum_g{h2}_{nh}")
                for h2 in range(H2)
                for nh in range(NLOC // FO)
            }
            for dk in range(DK):
                for h2 in range(H2):
                    for nh in range(NLOC // FO):
                        nc.tensor.matmul(
                            psum_g[(h2, nh)],
                            wg1t_sb[dk][:, h2 * P : (h2 + 1) * P],
                            xT_sb[dk][:, nh * FO : (nh + 1) * FO],
                            start=(dk == 0),
                            stop=(dk == DK - 1),
                        )
            for h2 in range(H2):
                for nh in range(NLOC // FO):
                    nc.scalar.activation(
                        out=hT_sb[h2][:, nh * FO : (nh + 1) * FO],
                        in_=psum_g[(h2, nh)],
                        func=mybir.ActivationFunctionType.Relu,
                        bias=bg1_sb[:, h2 : h2 + 1],
                    )

            # ---- gating: logits -> softmax -> gates (+ gates/32), gates.T ----
            # logits = (32h) @ (W_g2/32).T + b_g2 — exactly h @ W_g2.T + b_g2
            gates_sb = []
            gates32_sb = []
            gatesT_sb = []
            for nt in range(NT):
                psum_l = prop.tile([P, E], F32, tag="pro")
                for h2 in range(H2):
                    nc.tensor.matmul(
                        psum_l,
                        hT_sb[h2][:, nt * P : (nt + 1) * P],
                        wg2t_sb[:, h2, :],
                        start=(h2 == 0),
                        stop=False,
                    )
                nc.tensor.matmul(psum_l, ones_sb, bg2_sb, start=False, stop=True)

                negmax = workp.tile([P, 1], F32, tag="negmax")
                nc.vector.reduce_max(
                    negmax, psum_l, axis=mybir.AxisListType.X, negate=True
                )
                gates = constp.tile([P, E], F32, tag=f"gates{nt}", name=f"gates{nt}")
                sumexp = workp.tile([P, 1], F32, tag="sumexp")
                nc.scalar.activation(
                    out=gates,
                    in_=psum_l,
                    func=mybir.ActivationFunctionType.Exp,
                    bias=negmax,
                    accum_out=sumexp,
                )
                rsum = workp.tile([P, 1], F32, tag="rsum")
                nc.vector.reciprocal(rsum, sumexp)
                nc.vector.tensor_scalar_mul(gates, gates, rsum)
                gates32 = constp.tile(
                    [P, E], F32, tag=f"gates32{nt}", name=f"gates32{nt}"
                )
                nc.vector.tensor_scalar_mul(gates32, gates, 1.0 / SCALE)
                gates_sb.append(gates)
                gates32_sb.append(gates32)

            acc_sb = [
                [
                    constp.tile(
                        [P, FO], F32, tag=f"acc{nt}_{oh}", name=f"acc{nt}_{oh}"
                    )
                    for oh in range(OH)
                ]
                for nt in range(NT)
            ]

            # ---- main loop: stream experts, accumulate gate-weighted GEMM.
            # Per psum tile: 6 bf16 matmuls (dk 0-5) + 1 fp8 DoubleRow matmul
            # covering dk 6-7 at 2x rate. ----
            for e in range(E):
                if e == 0:
                    w_6, w_8 = w0_6, w0_8
                else:
                    w_6 = wpool.tile([P, DK6, O], BF16, tag="wh6", name=f"wh6_e{e}")
                    nc.sync.dma_start(out=w_6, in_=wt6[e])
                    w_8 = wpool.tile([P, 2, O], FP8, tag="wh8", name=f"wh8_e{e}")
                    nc.sync.dma_start(out=w_8, in_=w8[e])
                for oh in range(OH):
                    for nt in range(NT):
                        psum = mmp.tile([P, FO], F32, tag="mm")
                        for dk in range(DK6):
                            nc.tensor.matmul(
                                psum,
                                xT_sb[dk][:, nt * P : (nt + 1) * P],
                                w_6[:, dk, oh * FO : (oh + 1) * FO],
                                start=(dk == 0),
                                stop=False,
                            )
                        nc.tensor.matmul(
                            psum,
                            x8_sb[:, :, nt * P : (nt + 1) * P],
                            w_8[:, :, oh * FO : (oh + 1) * FO],
                            start=False,
                            stop=True,
                            perf_mode=mybir.MatmulPerfMode.DoubleRow,
                        )
                        acc = acc_sb[nt][oh]
                        if e == 0:
                            nc.scalar.mul(acc, psum, gates32_sb[nt][:, e : e + 1])
                        else:
                            tmp = workp.tile([P, FO], F32, tag="tmp", name="tmp")
                            nc.scalar.mul(tmp, psum, gates32_sb[nt][:, e : e + 1])
                            nc.vector.tensor_add(acc, acc, tmp)
                        if e == E - 1:
                            nc.scalar.dma_start(
                                out=out[
                                    nt * P : (nt + 1) * P, oh * FO : (oh + 1) * FO
                                ],
                                in_=acc,
                            )
                if e == 0:
                    # gates.T + bias matmuls — emitted here so the PE work
                    # hides inside experts 0-1's dense matmul stream and the
                    # kernel tail stays short
                    for nt in range(NT):
                        psum_t = prop.tile([E, P], F32, tag="pro", name="psum_t")
                        nc.tensor.transpose(psum_t, gates_sb[nt], ident_sb)
                        gatesT = constp.tile(
                            [E, P], BF16, tag=f"gatesT{nt}", name=f"gatesT{nt}"
                        )
                        nc.scalar.copy(out=gatesT, in_=psum_t)
                        gatesT_sb.append(gatesT)
                if e == 1:
                    for nt in range(NT):
                        for boh in range(OH):
                            psum_b = biasp.tile(
                                [P, FO], F32, tag="bias", name="psum_b"
                            )
                            nc.tensor.matmul(
                                psum_b,
                                gatesT_sb[nt],
                                be_sb[:, boh * FO : (boh + 1) * FO],
                                start=True,
                                stop=True,
                            )
                            nc.vector.tensor_add(
                                acc_sb[nt][boh], acc_sb[nt][boh], psum_b
                            )

    legalize_single_wait(nc)
    return nc


_NC_CACHE = {}


def _get_nc():
    if "nc" not in _NC_CACHE:
        _NC_CACHE["nc"] = build_moe()
    return _NC_CACHE["nc"]


def make_in_maps(x, W_e, b_e, W_g1, b_g1, W_g2, b_g2):
    x = np.asarray(x, dtype=np.float32)
    W_e = np.asarray(W_e, dtype=np.float32)
    # bf16 slabs: W_e[e] is [O, D]; take d < KCUT, lay out [p, j, o], d=j*128+p
    wt6 = np.ascontiguousarray(
        W_e[:, :, :KCUT]                       # [E, O, KCUT]
        .reshape(E, O, DK6, P)                 # d = j*128 + p
        .transpose(0, 3, 2, 1)                 # [E, P, DK6, O]
    ).astype(BF)
    # fp8 pair: d >= KCUT, scaled by 32, lay out [p, i, o], d=KCUT+i*128+p
    w8 = np.ascontiguousarray(
        (W_e[:, :, KCUT:] * SCALE)
        .reshape(E, O, 2, P)
        .transpose(0, 3, 2, 1)                 # [E, P, 2, O]
    ).astype(E4M3)
    wg1t = np.ascontiguousarray(np.asarray(W_g1, dtype=np.float32).T).astype(BF)
    wg2t = np.ascontiguousarray(
        np.asarray(W_g2, dtype=np.float32).T / SCALE
    ).astype(BF)
    bg1 = np.asarray(b_g1, dtype=np.float32) * SCALE
    bg2 = np.asarray(b_g2, dtype=np.float32).astype(BF)
    be = np.asarray(b_e, dtype=np.float32).astype(BF)
    ident_np = np.eye(P, dtype=np.float32)
    in_maps = []
    for c in range(NCORES):
        x_c = x[c * NLOC : (c + 1) * NLOC, :]
        xT_c = np.ascontiguousarray((x_c * SCALE).T.astype(BF))
        x8_c = np.ascontiguousarray(
            x_c[:, KCUT:]                      # [NLOC, 256]
            .reshape(NLOC, 2, P)
            .transpose(2, 1, 0)                # [P, 2, NLOC]
        ).astype(E4M3)
        in_maps.append(
            {
                "xT": xT_c,
                "x8": x8_c,
                "wt6": wt6,
                "w8": w8,
                "wg1t": wg1t,
                "wg2t": wg2t,
                "bg1": bg1,
                "bg2": bg2,
                "be": be,
                "ident": ident_np,
            }
        )
    return in_maps


def kernel(x, W_e, b_e, W_g1, b_g1, W_g2, b_g2, **run_kwargs):
    nc = _get_nc()
    in_maps = make_in_maps(x, W_e, b_e, W_g1, b_g1, W_g2, b_g2)
    res = run_bass_kernel_spmd(nc, in_maps, core_ids=list(range(NCORES)), **run_kwargs)
    out = np.concatenate([res.results[c]["out"] for c in range(NCORES)], axis=0)
    if run_kwargs:
        kernel.last_results = res
    return out


if __name__ == "__main__":
    rng = np.random.default_rng(0)
    s = 1.0 / np.sqrt(D)
    sh = 1.0 / np.sqrt(H)
    inputs = {
        "x": rng.standard_normal((N, D), dtype=np.float32),
        "W_e": rng.uniform(-s, s, (E, O, D)).astype(np.float32),
        "b_e": rng.uniform(-s, s, (E, O)).astype(np.float32),
        "W_g1": rng.uniform(-s, s, (H, D)).astype(np.float32),
        "b_g1": rng.uniform(-s, s, (H,)).astype(np.float32),
        "W_g2": rng.uniform(-sh, sh, (E, H)).astype(np.float32),
        "b_g2": rng.uniform(-sh, sh, (E,)).astype(np.float32),
    }
    out = kernel(**inputs)
    print("out", out.shape, out.dtype, float(np.abs(out).max()))


# revision 13
# speedup vs baseline: 1.0905x; 1.0181x over previous
"""Dense MoE (all-experts, gate-weighted sum) on 8 Trainium2 NeuronCores.

Sharding: pure data-parallel over the token axis N (8192 -> 1024 rows/core);
every core holds all 8 experts, so no collectives are needed.

Math folded per core (N_loc=1024, D=1024, E=8, O=1024, H=256):
    h      = relu(x @ W_g1.T + b_g1)                 # gating MLP
    gates  = softmax(h @ W_g2.T + b_g2)              # fp32 softmax
    out    = sum_e gates[:,e] * (x @ W_e[e].T) + gates @ b_e

Precision/speed hybrid: the expert GEMM contraction over D=1024 runs
dk 0-5 (768 rows) in bf16 and dk 6-7 (256 rows) as ONE fp8-e4m3
DoubleRow matmul (2x PE rate), all accumulating in the same fp32 PSUM
bank. To share one PSUM scale, x is pre-scaled by 32 for the bf16 path
(matching W8 = e4m3(32*W) on the fp8 path) and the gating network sees
the same 32x through an exact rescale: b_g1 *= 32 (relu is positively
homogeneous) and W_g2 /= 32, so logits/gates are unchanged. The gate
weighting uses gates/32 to undo the scale.

The bias term rides a tiny K=8 matmul (gates.T as stationary operand),
overlapped with expert 1's GEMM stream.

Input DMAs are split across the sync-engine and gpsimd-engine queues
(parallel hardware rings) with host-side swizzles giving 2-12KB
contiguous lines per partition, so the gating + expert-0 operands land
before the PE finishes its warm-up/gating phase (avoids the mid-kernel
HAM half-clock dip the serial-queue version hit).
"""

import numpy as np
import ml_dtypes

import concourse.bass as bass
import concourse.mybir as mybir
import concourse.tile as tile
from concourse.bass_utils import run_bass_kernel_spmd

N, D, E, O, H = 8192, 1024, 8, 1024, 256
NCORES = 8
NLOC = N // NCORES          # 1024 rows per core
P = 128                     # partitions
NT = NLOC // P              # 8 n-tiles
DK = D // P                 # 8 contraction tiles
DK6 = 6                     # bf16 contraction tiles (dk 0-5)
KCUT = DK6 * P              # 768: d >= KCUT handled by the fp8 pair
FO = 512                    # matmul moving free dim (one PSUM bank of fp32)
OH = O // FO                # 2 output halves
H2 = H // P                 # 2 h-tiles
SCALE = 32.0
HYBK = 6                    # experts 0..HYBK-1 use the fp8 pair; rest pure bf16
                            # (dials worst-case quantization error by sqrt(HYBK/8))
BF16 = mybir.dt.bfloat16
FP8 = mybir.dt.float8e4
F32 = mybir.dt.float32
BF = ml_dtypes.bfloat16
E4M3 = ml_dtypes.float8_e4m3
NWARM = 16


def legalize_single_wait(nc, max_waits=1):
    """This walrus build rejects instructions carrying more than one sync
    wait. Split each multi-wait instruction: excess waits move onto fresh
    same-engine NoOps inserted immediately before it (identical semantics:
    the engine stalls at the same program point on every semaphore)."""
    for f in nc.m.functions:
        for blk in f.blocks:
            insts = list(blk.instructions)
            if all(
                (i.sync_info is None or len(i.sync_info.on_wait) <= max_waits)
                for i in insts
            ):
                continue
            new = []
            for inst in insts:
                si = inst.sync_info
                if si is not None and len(si.on_wait) > max_waits:
                    waits = list(si.on_wait)
                    for k, w in enumerate(waits[:-max_waits]):
                        nop = mybir.InstNoOp(name=f"{inst.name}-w{k}")
                        nop.engine = inst.engine
                        nop.sync_info = mybir.SyncInfo(on_wait=[w], on_update=[])
                        new.append(nop)
                    si.on_wait = waits[-max_waits:]
                new.append(inst)
            blk.instructions = new
    return nc


def build_moe():
    nc = bass.Bass(target_bir_lowering=False)
    xT = nc.dram_tensor("xT", [D, NLOC], BF16, kind="ExternalInput")      # 32*x.T
    # fp8 pair operands laid out so every DoubleRow matmul slice is fully
    # contiguous (strided pair slices cost ~2x on the PE moving stream)
    x8 = nc.dram_tensor("x8", [P, NT, 2, P], FP8, kind="ExternalInput")   # x pair
    wt6 = nc.dram_tensor("wt6", [E, P, DK6, O], BF16, kind="ExternalInput")
    w8 = nc.dram_tensor("w8", [HYBK, P, OH, 2, FO], FP8, kind="ExternalInput")  # 32*W
    wtb = nc.dram_tensor("wtb", [E - HYBK, P, 2, O], BF16, kind="ExternalInput")
    wg1t = nc.dram_tensor("wg1t", [D, H], BF16, kind="ExternalInput")
    wg2t = nc.dram_tensor("wg2t", [H, E], BF16, kind="ExternalInput")     # /32
    bg1 = nc.dram_tensor("bg1", [H], F32, kind="ExternalInput")           # *32
    bg2 = nc.dram_tensor("bg2", [E], BF16, kind="ExternalInput")
    be = nc.dram_tensor("be", [E, O], BF16, kind="ExternalInput")
    ident = nc.dram_tensor("ident", [P, P], F32, kind="ExternalInput")
    out = nc.dram_tensor("out", [NLOC, O], F32, kind="ExternalOutput")

    with tile.TileContext(nc) as tc:
        with (
            tc.tile_pool(name="const", bufs=1) as constp,
            tc.tile_pool(name="wpool", bufs=4) as wpool,
            tc.tile_pool(name="work", bufs=4) as workp,
            tc.tile_pool(name="pro_ps", bufs=2, space="PSUM") as prop,
            tc.tile_pool(name="bias_ps", bufs=1, space="PSUM") as biasp,
            tc.tile_pool(name="mm_ps", bufs=5, space="PSUM") as mmp,
        ):
            # ---- PE warm-up: dummy matmuls on memset tiles (no DMA deps)
            # keep the PE busy while the first transfers land, so the HAM
            # clock-gate reaches 2.4 GHz before real work arrives ----
            warm_a = constp.tile([P, P], BF16, tag="warm_a")
            nc.vector.memset(warm_a, 0.0)
            warm_b = constp.tile([P, FO], BF16, tag="warm_b")
            nc.vector.memset(warm_b, 0.0)
            for i in range(NWARM):
                wpsum = mmp.tile([P, FO], F32, tag="mm", name=f"warm{i}")
                nc.tensor.matmul(wpsum, warm_a, warm_b, start=True, stop=True)

            # ---- resident inputs. Gating-critical chunks (wg1t, xT) are
            # interleaved per-dk and split across BOTH dma queues (sync gets
            # even dk, gpsimd odd dk) so the gating matmul stream is paced at
            # twice the single-queue delivery rate. Expert-0 weights ride the
            # gpsimd queue, experts 1-7 the sync queue. ----
            wg1t_sb = [
                constp.tile([P, H], BF16, tag=f"wg1t{dk}", name=f"wg1t{dk}")
                for dk in range(DK)
            ]
            xT_sb = [
                constp.tile([P, NLOC], BF16, tag=f"xTd{dk}", name=f"xTd{dk}")
                for dk in range(DK)
            ]
            for dk in range(0, DK, 2):
                nc.sync.dma_start(
                    out=wg1t_sb[dk], in_=wg1t[dk * P : (dk + 1) * P, :]
                )
                nc.gpsimd.dma_start(
                    out=wg1t_sb[dk + 1], in_=wg1t[(dk + 1) * P : (dk + 2) * P, :]
                )
                nc.sync.dma_start(out=xT_sb[dk], in_=xT[dk * P : (dk + 1) * P, :])
                nc.gpsimd.dma_start(
                    out=xT_sb[dk + 1], in_=xT[(dk + 1) * P : (dk + 2) * P, :]
                )
            # expert-0 weights + fp8 x on the fast sync queue, right behind
            # the even gating chunks and AHEAD of experts 1-7 (the gpsimd
            # queue drains at well under half the sync queue's rate)
            w0_6 = wpool.tile([P, DK6, O], BF16, tag="wh6", name="wh6_e0")
            nc.sync.dma_start(out=w0_6, in_=wt6[0])
            w0_8 = wpool.tile([P, OH, 2, FO], FP8, tag="wh8", name="wh8_e0")
            nc.sync.dma_start(out=w0_8, in_=w8[0])
            x8_sb = constp.tile([P, NT, 2, P], FP8, tag="x8")
            nc.sync.dma_start(out=x8_sb, in_=x8[:, :, :, :])
            # small gating/bias constants (tiny, needed ~20us in)
            wg2t_sb = constp.tile([P, H2, E], BF16, tag="wg2t")
            nc.gpsimd.dma_start(
                out=wg2t_sb, in_=wg2t.rearrange("(h2 p) e -> p h2 e", p=P)
            )
            bg1_sb = constp.tile([P, H2], F32, tag="bg1")
            nc.gpsimd.dma_start(out=bg1_sb, in_=bg1.rearrange("(h2 p) -> p h2", p=P))
            bg2_sb = constp.tile([1, E], BF16, tag="bg2")
            nc.gpsimd.dma_start(out=bg2_sb, in_=bg2[:])
            ident_sb = constp.tile([P, P], F32, tag="ident")
            nc.gpsimd.dma_start(out=ident_sb, in_=ident[:, :])
            be_sb = constp.tile([E, O], BF16, tag="be")
            nc.gpsimd.dma_start(out=be_sb, in_=be[:, :])
            ones_sb = constp.tile([1, P], BF16, tag="ones")
            nc.vector.memset(ones_sb, 1.0)

            # ---- gating: hT[h, n] = relu(W_g1 @ (32x).T + 32*b_g1) = 32*h ----
            hT_sb = [
                constp.tile([P, NLOC], BF16, tag=f"hT{h2}", name=f"hT{h2}")
                for h2 in range(H2)
            ]
            psum_g = {
                (h2, nh): mmp.tile([P, FO], F32, tag="mm", name=f"psum_g{h2}_{nh}")
                for h2 in range(H2)
                for nh in range(NLOC // FO)
            }
            for dk in range(DK):
                for h2 in range(H2):
                    for nh in range(NLOC // FO):
                        nc.tensor.matmul(
                            psum_g[(h2, nh)],
                            wg1t_sb[dk][:, h2 * P : (h2 + 1) * P],
                            xT_sb[dk][:, nh * FO : (nh + 1) * FO],
                            start=(dk == 0),
                            stop=(dk == DK - 1),
                        )
            for h2 in range(H2):
                for nh in range(NLOC // FO):
                    nc.scalar.activation(
                        out=hT_sb[h2][:, nh * FO : (nh + 1) * FO],
                        in_=psum_g[(h2, nh)],
                        func=mybir.ActivationFunctionType.Relu,
                        bias=bg1_sb[:, h2 : h2 + 1],
                    )

            # ---- gating: logits -> softmax -> gates (+ gates/32), gates.T ----
            # logits = (32h) @ (W_g2/32).T + b_g2 — exactly h @ W_g2.T + b_g2
            gates_sb = []
            gates32_sb = []
            gatesT_sb = []
            for nt in range(NT):
                psum_l = prop.tile([P, E], F32, tag="pro")
                for h2 in range(H2):
                    nc.tensor.matmul(
                        psum_l,
                        hT_sb[h2][:, nt * P : (nt + 1) * P],
                        wg2t_sb[:, h2, :],
                        start=(h2 == 0),
                        stop=False,
                    )
                nc.tensor.matmul(psum_l, ones_sb, bg2_sb, start=False, stop=True)

                negmax = workp.tile([P, 1], F32, tag="negmax")
                nc.vector.reduce_max(
                    negmax, psum_l, axis=mybir.AxisListType.X, negate=True
                )
                gates = constp.tile([P, E], F32, tag=f"gates{nt}", name=f"gates{nt}")
                sumexp = workp.tile([P, 1], F32, tag="sumexp")
                nc.scalar.activation(
                    out=gates,
                    in_=psum_l,
                    func=mybir.ActivationFunctionType.Exp,
                    bias=negmax,
                    accum_out=sumexp,
                )
                rsum = workp.tile([P, 1], F32, tag="rsum")
                nc.vector.reciprocal(rsum, sumexp)
                nc.vector.tensor_scalar_mul(gates, gates, rsum)
                gates32 = constp.tile(
                    [P, E], F32, tag=f"gates32{nt}", name=f"gates32{nt}"
                )
                nc.vector.tensor_scalar_mul(gates32, gates, 1.0 / SCALE)
                gates_sb.append(gates)
                gates32_sb.append(gates32)

            acc_sb = [
                [
                    constp.tile(
                        [P, FO], F32, tag=f"acc{nt}_{oh}", name=f"acc{nt}_{oh}"
                    )
                    for oh in range(OH)
                ]
                for nt in range(NT)
            ]

            # ---- main loop: stream experts, accumulate gate-weighted GEMM.
            # Per psum tile: 6 bf16 matmuls (dk 0-5) + 1 fp8 DoubleRow matmul
            # covering dk 6-7 at 2x rate. ----
            for e in range(E):
                hyb = e < HYBK
                if e == 0:
                    w_6, w_8 = w0_6, w0_8
                    w_b = None
                else:
                    w_6 = wpool.tile([P, DK6, O], BF16, tag="wh6", name=f"wh6_e{e}")
                    nc.sync.dma_start(out=w_6, in_=wt6[e])
                    if hyb:
                        w_8 = wpool.tile(
                            [P, OH, 2, FO], FP8, tag="wh8", name=f"wh8_e{e}"
                        )
                        nc.sync.dma_start(out=w_8, in_=w8[e])
                    else:
                        w_b = wpool.tile([P, 2, O], BF16, tag="whb", name=f"whb_e{e}")
                        nc.sync.dma_start(out=w_b, in_=wtb[e - HYBK])
                for oh in range(OH):
                    for nt in range(NT):
                        psum = mmp.tile([P, FO], F32, tag="mm")
                        for dk in range(DK6):
                            nc.tensor.matmul(
                                psum,
                                xT_sb[dk][:, nt * P : (nt + 1) * P],
                                w_6[:, dk, oh * FO : (oh + 1) * FO],
                                start=(dk == 0),
                                stop=False,
                            )
                        if hyb:
                            nc.tensor.matmul(
                                psum,
                                x8_sb[:, nt, :, :],
                                w_8[:, oh, :, :],
                                start=False,
                                stop=True,
                                perf_mode=mybir.MatmulPerfMode.DoubleRow,
                            )
                        else:
                            for i in range(2):
                                nc.tensor.matmul(
                                    psum,
                                    xT_sb[DK6 + i][:, nt * P : (nt + 1) * P],
                                    w_b[:, i, oh * FO : (oh + 1) * FO],
                                    start=False,
                                    stop=(i == 1),
                                )
                        acc = acc_sb[nt][oh]
                        if e == 0:
                            nc.scalar.mul(acc, psum, gates32_sb[nt][:, e : e + 1])
                        else:
                            tmp = workp.tile([P, FO], F32, tag="tmp", name="tmp")
                            nc.scalar.mul(tmp, psum, gates32_sb[nt][:, e : e + 1])
                            nc.vector.tensor_add(acc, acc, tmp)
                        if e == E - 1:
                            nc.scalar.dma_start(
                                out=out[
                                    nt * P : (nt + 1) * P, oh * FO : (oh + 1) * FO
                                ],
                                in_=acc,
                            )
                if e == 0:
                    # gates.T + bias matmuls — emitted here so the PE work
                    # hides inside experts 0-1's dense matmul stream and the
                    # kernel tail stays short
                    for nt in range(NT):
                        psum_t = prop.tile([E, P], F32, tag="pro", name="psum_t")
                        nc.tensor.transpose(psum_t, gates_sb[nt], ident_sb)
                        gatesT = constp.tile(
                            [E, P], BF16, tag=f"gatesT{nt}", name=f"gatesT{nt}"
                        )
                        nc.scalar.copy(out=gatesT, in_=psum_t)
                        gatesT_sb.append(gatesT)
                if e == 1:
                    for nt in range(NT):
                        for boh in range(OH):
                            psum_b = biasp.tile(
                                [P, FO], F32, tag="bias", name="psum_b"
                            )
                            nc.tensor.matmul(
                                psum_b,
                                gatesT_sb[nt],
                                be_sb[:, boh * FO : (boh + 1) * FO],
                                start=True,
                                stop=True,
                            )
                            nc.vector.tensor_add(
                                acc_sb[nt][boh], acc_sb[nt][boh], psum_b
                            )

    legalize_single_wait(nc)
    return nc


_NC_CACHE = {}


def _get_nc():
    if "nc" not in _NC_CACHE:
        _NC_CACHE["nc"] = build_moe()
    return _NC_CACHE["nc"]


def make_in_maps(x, W_e, b_e, W_g1, b_g1, W_g2, b_g2):
    x = np.asarray(x, dtype=np.float32)
    W_e = np.asarray(W_e, dtype=np.float32)
    # bf16 slabs: W_e[e] is [O, D]; take d < KCUT, lay out [p, j, o], d=j*128+p
    wt6 = np.ascontiguousarray(
        W_e[:, :, :KCUT]                       # [E, O, KCUT]
        .reshape(E, O, DK6, P)                 # d = j*128 + p
        .transpose(0, 3, 2, 1)                 # [E, P, DK6, O]
    ).astype(BF)
    # fp8 pair: d >= KCUT, scaled by 32, laid out [p, oh, i, fo] so the
    # DoubleRow rhs slice [P, 2, FO] is contiguous per output half
    w8 = np.ascontiguousarray(
        (W_e[:HYBK, :, KCUT:] * SCALE)
        .reshape(HYBK, OH, FO, 2, P)           # o = oh*FO+fo, d = KCUT+i*128+p
        .transpose(0, 4, 1, 3, 2)              # [HYBK, P, OH, 2, FO]
    ).astype(E4M3)
    # bf16 dk 6-7 slabs for the non-hybrid experts
    wtb = np.ascontiguousarray(
        W_e[HYBK:, :, KCUT:]
        .reshape(E - HYBK, O, 2, P)
        .transpose(0, 3, 2, 1)                 # [E-HYBK, P, 2, O]
    ).astype(BF)
    wg1t = np.ascontiguousarray(np.asarray(W_g1, dtype=np.float32).T).astype(BF)
    wg2t = np.ascontiguousarray(
        np.asarray(W_g2, dtype=np.float32).T / SCALE
    ).astype(BF)
    bg1 = np.asarray(b_g1, dtype=np.float32) * SCALE
    bg2 = np.asarray(b_g2, dtype=np.float32).astype(BF)
    be = np.asarray(b_e, dtype=np.float32).astype(BF)
    ident_np = np.eye(P, dtype=np.float32)
    in_maps = []
    for c in range(NCORES):
        x_c = x[c * NLOC : (c + 1) * NLOC, :]
        xT_c = np.ascontiguousarray((x_c * SCALE).T.astype(BF))
        x8_c = np.ascontiguousarray(
            x_c[:, KCUT:]                      # [NLOC, 256]
            .reshape(NT, P, 2, P)              # [nt, m, i, p]
            .transpose(3, 0, 2, 1)             # [P, NT, 2, P(m)]
        ).astype(E4M3)
        in_maps.append(
            {
                "xT": xT_c,
                "x8": x8_c,
                "wt6": wt6,
                "w8": w8,
                "wtb": wtb,
                "wg1t": wg1t,
                "wg2t": wg2t,
                "bg1": bg1,
                "bg2": bg2,
                "be": be,
                "ident": ident_np,
            }
        )
    return in_maps


def kernel(x, W_e, b_e, W_g1, b_g1, W_g2, b_g2, **run_kwargs):
    nc = _get_nc()
    in_maps = make_in_maps(x, W_e, b_e, W_g1, b_g1, W_g2, b_g2)
    res = run_bass_kernel_spmd(nc, in_maps, core_ids=list(range(NCORES)), **run_kwargs)
    out = np.concatenate([res.results[c]["out"] for c in range(NCORES)], axis=0)
    if run_kwargs:
        kernel.last_results = res
    return out


if __name__ == "__main__":
    rng = np.random.default_rng(0)
    s = 1.0 / np.sqrt(D)
    sh = 1.0 / np.sqrt(H)
    inputs = {
        "x": rng.standard_normal((N, D), dtype=np.float32),
        "W_e": rng.uniform(-s, s, (E, O, D)).astype(np.float32),
        "b_e": rng.uniform(-s, s, (E, O)).astype(np.float32),
        "W_g1": rng.uniform(-s, s, (H, D)).astype(np.float32),
        "b_g1": rng.uniform(-s, s, (H,)).astype(np.float32),
        "W_g2": rng.uniform(-sh, sh, (E, H)).astype(np.float32),
        "b_g2": rng.uniform(-sh, sh, (E,)).astype(np.float32),
    }
    out = kernel(**inputs)
    print("out", out.shape, out.dtype, float(np.abs(out).max()))
